# revision 10
# baseline (speedup 1.0000x reference)
"""Trainium2 Bass kernel for nn_DistinctivenessLoss.

Math: with unit-normalized 48-dim descriptors, dist(a,b) < 1  <=>  <a,b> > 0.5,
so each image's two 4096x4096 cdist+count reductions collapse to ONE 4096x4096
gram matrix S = d1^T d2 per image:
  mx1[i] = #{j : S[i,j] > 0.5}          (row counts)
  mx2[i] = colcount[flat2[i]],  colcount[j] = #{i : S[i,j] > 0.5}

Sharding: 2 cores per image (B=4, 8 cores); each core computes 2048 rows of S.
On-device per core:
  - gram matmuls in bf16 (inputs rounded to bf16; fp32 PSUM accumulate --
    measured end-to-end loss impact ~4e-5 relative), row-tiled 2x (K=48<=64)
    at tile positions (0,0)/(64,0)
  - threshold pass split between ScalarE (Sign activation -> fp8 +-1, fused
    row-sum via accum_out) and VectorE (is_gt -> fp8 {0,1}, fused row-sum)
    over 2-bank [128,1024] PSUM supertiles
  - column counts via ONE fp8 DoubleRow "selector" matmul per pair of sign
    tiles (uniform 0.5 weights route chunk c's colsum into PSUM row c),
    accumulated in a single PSUM bank
  - the peaky-loss max-pool path for one 64x64 attention map (3x3 avg pool as
    two banded matmuls, 33x33 max pool as log-step max chains + 32x32 DVE
    transposes); the avg33 branch folds algebraically into a host-side
    bilinear form u^T X u (sum of avg pools == bilinear form in the input)
Host: input normalization/layout, count decode, gathers, and the O(n) loss
epilogue (exact fp32/f64, negligible work).
"""

import numpy as np
import ml_dtypes

B, C, H, W = 4, 48, 64, 64
N = H * W                  # 4096
NCORES = 8
ROWS = N // 2              # rows of S per core
NT = ROWS // 128           # 16 row-tiles per core
NCH = N // 512             # 8 column chunks
NSC = NCH // 2             # 4 superchunks (2-bank PSUM supertiles)
THR = 0.5
TAU = 0.25
LAMBDA_PEAKY = 0.2
NPOOL = 32
NEG_INF = float("-inf")

# (row-tile, superchunk) -> engine class: 0=ACT/sign(+-1), 1=DVE/{0,2}.
ASSIGN = np.fromfunction(lambda t, s: (t + s) % 2, (NT, NSC), dtype=np.int64)
N_ACT_PER_CHUNK = np.array([(ASSIGN[:, c // 2] == 0).sum() for c in range(NCH)])
N_ACT_PER_TILE = (ASSIGN == 0).sum(axis=1)    # [NT], in superchunk units

_PROGRAM = None


def _build_program():
    from contextlib import ExitStack

    import concourse.bass as bass
    import concourse.mybir as mybir
    import concourse.tile as tile
    from concourse import bacc

    F32 = mybir.dt.float32
    BF16 = mybir.dt.bfloat16
    FP8 = mybir.dt.float8e4
    Sign = mybir.ActivationFunctionType.Sign
    DR = mybir.MatmulPerfMode.DoubleRow
    ts, ds = bass.ts, bass.ds

    nc = bacc.Bacc("TRN2", target_bir_lowering=False, debug=False,
                   num_devices=NCORES)

    d1w_d = nc.dram_tensor("d1w", [128, NT * 64], BF16, kind="ExternalInput").ap()
    d2d_d = nc.dram_tensor("d2d", [128, N], BF16, kind="ExternalInput").ap()
    wsel_d = nc.dram_tensor("wsel", [128, NCH * 256], FP8,
                            kind="ExternalInput").ap()
    attn_d = nc.dram_tensor("attn", [64, 64], F32, kind="ExternalInput").ap()
    pk_d = nc.dram_tensor("pk", [64, 64], F32, kind="ExternalInput").ap()

    rowA_d = nc.dram_tensor("rowA", [128, NT], F32, kind="ExternalOutput").ap()
    rowD_d = nc.dram_tensor("rowD", [128, NT], F32, kind="ExternalOutput").ap()
    cols_d = nc.dram_tensor("cols", [NCH, 512], F32, kind="ExternalOutput").ap()
    pkout_d = nc.dram_tensor("pkout", [64, 1], F32, kind="ExternalOutput").ap()

    with tile.TileContext(nc) as tc, ExitStack() as ctx:
        sb = ctx.enter_context(tc.tile_pool(name="sb", bufs=1))
        sgn_pool = ctx.enter_context(tc.tile_pool(name="sgn", bufs=3))

        # ---- inputs: peaky constants first (tiny; unblock the early peaky
        # matmuls), then gram operands in first-use order ----
        pk = sb.tile([64, 64], F32)
        nc.sync.dma_start(pk[:], pk_d)
        attn = sb.tile([64, 64], F32)
        nc.sync.dma_start(attn[:], attn_d)

        d1w = sb.tile([128, NT * 64], BF16)
        d2d = sb.tile([128, N], BF16)
        nc.sync.dma_start(d1w[:, 0:128], d1w_d[:, 0:128])
        nc.sync.dma_start(d2d[:, 0:1024], d2d_d[:, 0:1024])
        nc.sync.dma_start(d1w[:, 128:1024], d1w_d[:, 128:1024])
        nc.sync.dma_start(d2d[:, 1024:2048], d2d_d[:, 1024:2048])
        nc.sync.dma_start(d2d[:, 2048:4096], d2d_d[:, 2048:4096])
        wsel = sb.tile([128, NCH * 256], FP8)
        nc.sync.dma_start(wsel[:], wsel_d)

        nthr = sb.tile([128, 1], F32)
        nc.vector.memset(nthr[:], -THR)

        rowaccA = sb.tile([128, NT * NSC], F32)
        rowaccD = sb.tile([128, NT * NSC], F32)
        nc.vector.memset(rowaccA[:], 0.0)
        nc.vector.memset(rowaccD[:], 0.0)

        # ---- PE warm-up: dummy matmuls with no DMA dependencies run during
        # the input-DMA window and start the HAM activity window early ----
        pk_pool = ctx.enter_context(
            tc.tile_pool(name="pkps", bufs=1, space="PSUM"))
        warm_w = sb.tile([64, 128], BF16)
        warm_x = sb.tile([64, 512], BF16)
        nc.gpsimd.memset(warm_w[:], 1.0)
        nc.gpsimd.memset(warm_x[:], 1.0)
        warm_ps = pk_pool.tile([128, 512], F32, tag="pk")
        for _ in range(5):
            nc.tensor.matmul(warm_ps[:], warm_w[0:64, :], warm_x[0:64, :],
                             start=True, stop=True)

        gram_pool = ctx.enter_context(tc.tile_pool(name="gram", bufs=1,
                                                   space="PSUM"))
        cs_pool = ctx.enter_context(tc.tile_pool(name="cs", bufs=1, space="PSUM"))
        colacc = cs_pool.tile([128, 512], F32, tag="ca")

        # peaky state built incrementally between gram iterations so the
        # serial PE->ACT->PE chain never stalls an engine queue
        pk_state = {}

        def peaky_step(step):
            if step == 0:      # p1 = X^T B3 (PE, after warmup, pre-main)
                p1 = pk_pool.tile([128, 512], F32, tag="pk")
                nc.tensor.matmul(p1[0:64, 0:64], attn[:], pk[:],
                                 start=True, stop=True)
                pk_state["p1"] = p1
            elif step == 1:    # ACT's first op; p1 ready ~2.5us
                s1 = sb.tile([64, 64], F32, tag="s1")
                nc.scalar.copy(s1[:], pk_state["p1"][0:64, 0:64])
                pk_state["s1"] = s1
            elif step == 2:    # p2 = s1^T B3 = B3 X B3 = sali (PE)
                p2 = pk_pool.tile([128, 512], F32, tag="pk")
                nc.tensor.matmul(p2[0:64, 0:64], pk_state["s1"][:], pk[:],
                                 start=True, stop=True)
                pk_state["p2"] = p2
            elif step == 3:
                sali = sb.tile([64, 64], F32, tag="sali")
                nc.scalar.copy(sali[:], pk_state["p2"][0:64, 0:64])
                pk_state["sali"] = sali
            elif step == 4:
                with nc.named_scope("peaky"):
                    sali = pk_state["sali"]

                    def max_chain(src_ap, tag):
                        pb = sb.tile([64, 128], F32, tag=tag + "pb")
                        nc.gpsimd.memset(pb[:, 0:16], NEG_INF)
                        nc.gpsimd.memset(pb[:, 80:112], NEG_INF)
                        nc.scalar.copy(pb[:, 16:80], src_ap)
                        cur = pb
                        for k in (1, 2, 4, 8, 16):
                            nxt = sb.tile([64, 128], F32, tag=tag + "s%d" % k)
                            nc.gpsimd.memset(nxt[:, 96:112], NEG_INF)
                            nc.vector.tensor_max(nxt[:, 0:96], cur[:, 0:96],
                                                 cur[:, k:96 + k])
                            cur = nxt
                        out = sb.tile([64, 64], F32, tag=tag + "o")
                        nc.vector.tensor_max(out[:], cur[:, 0:64], pb[:, 32:96])
                        return out

                    m1 = max_chain(sali[:], "mA")              # pooled along w
                    m1t = sb.tile([64, 64], F32, tag="m1t")
                    for bi in range(2):
                        for bj in range(2):
                            nc.vector.transpose(
                                m1t[ds(32 * bi, 32), ds(32 * bj, 32)],
                                m1[ds(32 * bj, 32), ds(32 * bi, 32)])
                    m2 = max_chain(m1t[:], "mB")               # pooled along h
                    gapm = sb.tile([64, 1], F32)
                    nc.vector.reduce_sum(gapm[:], m2[:],
                                         axis=mybir.AxisListType.X)
                    nc.sync.dma_start(pkout_d, gapm[:])

        peaky_step(0)
        peaky_step(1)

        # ---- main loop: gram + threshold + DoubleRow colsum, software-
        # pipelined: superchunk sc's colsum matmuls are emitted while sc+1's
        # grams stream, so the PE always has independent work queued ----
        ncs = [0]
        n_cs = NCH * NT // 2          # 64 DoubleRow matmuls total

        def colsum_mm(sc, cc, q, sgn):
            c = 2 * sc + cc
            w = wsel[:, ds(c * 256, 256)].rearrange(
                "p (two m) -> p two m", two=2)
            rhs = sgn[:].rearrange("p (t x) -> p t x", t=NT)[
                :, ds(2 * q, 2), ds(cc * 512, 512)]
            ncs[0] += 1
            nc.tensor.matmul(colacc[:], w, rhs,
                             start=(ncs[0] == 1), stop=(ncs[0] == n_cs),
                             perf_mode=DR, skip_group_check=True)

        peaky_after = {1: 2, 5: 3, 6: 4}   # global p-iter -> peaky step

        with nc.named_scope("main"):
            pending = None   # (sc, sgn) whose 16 colsum MMs are owed
            it = 0
            for sc in range(NSC):
                c0, c1 = 2 * sc, 2 * sc + 1
                sgn = sgn_pool.tile([128, NT * 1024], FP8, tag="sgn")
                for p in range(NT // 2):
                    g0 = gram_pool.tile([128, 1024], F32,
                                        tag="g%d" % ((2 * p) % 3))
                    g1 = gram_pool.tile([128, 1024], F32,
                                        tag="g%d" % ((2 * p + 1) % 3))
                    wl = d1w[0:C, ts(p, 128)]
                    wh = d1w[64:64 + C, ts(p, 128)]
                    nc.tensor.matmul(g0[:, 0:512], wl, d2d[0:C, ts(c0, 512)],
                                     start=True, stop=True)
                    nc.tensor.matmul(g0[:, 512:1024], wl, d2d[0:C, ts(c1, 512)],
                                     start=True, stop=True)
                    nc.tensor.matmul(g1[:, 0:512], wh,
                                     d2d[64:64 + C, ts(c0, 512)],
                                     start=True, stop=True)
                    nc.tensor.matmul(g1[:, 512:1024], wh,
                                     d2d[64:64 + C, ts(c1, 512)],
                                     start=True, stop=True)
                    for t, g in ((2 * p, g0), (2 * p + 1, g1)):
                        s = sgn[:, ts(t, 1024)]
                        if ASSIGN[t, sc] == 0:
                            nc.scalar.activation(
                                s, g[:], Sign, bias=nthr[:], scale=1.0,
                                accum_out=rowaccA[:, ds(t * NSC + sc, 1)])
                        else:
                            nc.vector.tensor_scalar(
                                s, g[:], THR, None,
                                mybir.AluOpType.is_gt, mybir.AluOpType.add,
                                accum_out=rowaccD[:, ds(t * NSC + sc, 1)])
                    if pending is not None:
                        psc, psgn = pending
                        # 2 of the 16 owed MMs per iteration, cc-major so each
                        # 256-col weight block is reused 8x back-to-back
                        for k in (2 * p, 2 * p + 1):
                            colsum_mm(psc, k // 8, k % 8, psgn)
                    if it in peaky_after:
                        peaky_step(peaky_after[it])
                    it += 1
                pending = (sc, sgn)
            psc, psgn = pending
            for k in range(16):
                colsum_mm(psc, k // 8, k % 8, psgn)

        # ---- row-count reduction + export ----
        redA = sb.tile([128, NT], F32)
        redD = sb.tile([128, NT], F32)
        nc.vector.reduce_sum(
            redA[:], rowaccA[:].rearrange("p (t c) -> p t c", t=NT, c=NSC),
            axis=mybir.AxisListType.X)
        nc.vector.reduce_sum(
            redD[:], rowaccD[:].rearrange("p (t c) -> p t c", t=NT, c=NSC),
            axis=mybir.AxisListType.X)
        nc.sync.dma_start(rowA_d, redA[:])
        nc.sync.dma_start(rowD_d, redD[:])

        # ---- column-count export: single accumulator bank, rows 0..7 ----
        cols_sb = sb.tile([NCH, 512], F32)
        nc.scalar.copy(cols_sb[:], colacc[0:NCH, :])
        nc.sync.dma_start(cols_d, cols_sb[:])

    nc.compile()
    return nc


def _get_program():
    global _PROGRAM
    if _PROGRAM is None:
        _PROGRAM = _build_program()
    return _PROGRAM


def _normalize(x):
    n = np.sqrt((x * x).sum(axis=0, keepdims=True, dtype=np.float32))
    return (x / np.maximum(n, np.float32(1e-12))).astype(np.float32)


def _make_consts():
    idx = np.arange(64)
    b3 = (np.abs(idx[:, None] - idx[None, :]) <= 1).astype(np.float32) / \
        np.float32(3.0)
    b33 = (np.abs(idx[:, None] - idx[None, :]) <= 16).astype(np.float64) / \
        np.float64(33.0)
    # sum(avg33(sali)) == u^T X u with u = (B33 @ B3).sum(axis=0)
    u = (b33 @ b3.astype(np.float64)).sum(axis=0)          # [64] float64

    # DoubleRow weight per chunk c: plane i multiplies sign tile 2q+i of each
    # pair.  ASSIGN[t, sc] = (t+sc)%2 alternates engine class inside every
    # pair, so the ACT plane (sign, +-1) gets 0.5 and the DVE plane (is_gt,
    # {0,1}) gets 1.0 -- which plane is which depends only on sc = c//2.
    wsel = np.zeros((128, NCH * 256), np.float32)
    for c in range(NCH):
        sc = c // 2
        w_plane0 = 0.5 if sc % 2 == 0 else 1.0   # tile 2q class = sc%2
        w_plane1 = 1.0 if sc % 2 == 0 else 0.5
        wsel[:, c * 256 + c] = w_plane0
        wsel[:, c * 256 + 128 + c] = w_plane1
    return b3, u, wsel.astype(ml_dtypes.float8_e4m3)


def _prepare_in_maps(x1, x2, a1, a2):
    b3, _, wsel = _make_consts()
    in_maps = []
    for b in range(B):
        d1 = _normalize(x1[b].reshape(C, N).astype(np.float32)) \
            .astype(ml_dtypes.bfloat16)
        d2 = _normalize(x2[b].reshape(C, N).astype(np.float32)) \
            .astype(ml_dtypes.bfloat16)
        d2d = np.zeros((128, N), ml_dtypes.bfloat16)
        d2d[0:C] = d2
        d2d[64:64 + C] = d2
        for half in range(2):
            base = half * ROWS
            d1w = np.zeros((128, NT * 64), ml_dtypes.bfloat16)
            for p in range(NT // 2):
                d1w[0:C, p * 128:(p + 1) * 128] = \
                    d1[:, base + (2 * p) * 128: base + (2 * p + 1) * 128]
                d1w[64:64 + C, p * 128:(p + 1) * 128] = \
                    d1[:, base + (2 * p + 1) * 128: base + (2 * p + 2) * 128]
            attn = (a1 if half == 0 else a2)[b, 0].astype(np.float32)
            in_maps.append({
                "d1w": d1w, "d2d": d2d, "wsel": np.asarray(wsel),
                "attn": np.ascontiguousarray(attn), "pk": b3,
            })
    return in_maps


def _postprocess(results, x1, x2, a1, a2, pos2):
    f32 = np.float32
    _, u, _ = _make_consts()
    mx1_halves = []
    colparts = []
    gap_means = np.zeros((NCORES,), np.float64)
    for core, r in enumerate(results):
        rowA = r["rowA"].astype(np.float32)   # [128, NT] sum of +-1 per tile
        rowD = r["rowD"].astype(np.float32)   # [128, NT] sum of {0,1}
        cnt = (rowA + 1024.0 * N_ACT_PER_TILE[None, :]) * 0.5 + rowD
        mx1_halves.append(cnt.T.reshape(ROWS))  # row t*128+p
        cols = r["cols"].astype(np.float32)     # [NCH, 512]
        colpart = cols + 64.0 * N_ACT_PER_CHUNK[:, None]
        colparts.append(colpart.reshape(N))
        # peaky: mean gap = (sum max33(sali) - u^T X u) / 4096
        b = core // 2
        attn = (a1 if core % 2 == 0 else a2)[b, 0].astype(np.float64)
        uxu = u @ attn @ u
        m2sum = float(r["pkout"].astype(np.float64).sum())
        gap_means[core] = (m2sum - uxu) / float(N)

    loss_imgs = np.zeros((B,), np.float32)
    for b in range(B):
        mx1 = np.concatenate([mx1_halves[2 * b], mx1_halves[2 * b + 1]])
        colcnt = colparts[2 * b] + colparts[2 * b + 1]
        flat2 = (pos2[b, 0].astype(np.int64) * W +
                 pos2[b, 1].astype(np.int64))
        mx2 = colcnt[flat2]
        scores1 = a1[b].reshape(N).astype(np.float32)
        scores2 = a2[b].reshape(N).astype(np.float32)[flat2]
        t1 = (f32(1.0) / (f32(1.0) + mx1.astype(np.float32)) **
              f32(TAU)).astype(np.float32)
        t2 = (f32(1.0) / (f32(1.0) + mx2.astype(np.float32)) **
              f32(TAU)).astype(np.float32)
        loss_imgs[b] = (np.abs(scores1 - t1).mean(dtype=np.float32) +
                        np.abs(scores2 - t2).mean(dtype=np.float32))

    loss = loss_imgs.mean(dtype=np.float32)
    pk1 = max(f32(0.0), f32(1.0) - f32(gap_means[0::2].mean()))
    pk2 = max(f32(0.0), f32(1.0) - f32(gap_means[1::2].mean()))
    loss = loss + f32(LAMBDA_PEAKY) * (pk1 + pk2) / f32(2.0)
    return np.asarray(loss, dtype=np.float32)


def _run(x1_encoded, x2_encoded, attentions1, attentions2, fmap_pos2,
         trace=False, trace_cores=None):
    from concourse import bass_utils

    nc = _get_program()
    in_maps = _prepare_in_maps(np.asarray(x1_encoded), np.asarray(x2_encoded),
                               np.asarray(attentions1),
                               np.asarray(attentions2))
    res = bass_utils.run_bass_kernel_spmd(
        nc, in_maps, core_ids=list(range(NCORES)), trace=trace,
        trace_cores=trace_cores)
    loss = _postprocess(res.results, np.asarray(x1_encoded),
                        np.asarray(x2_encoded), np.asarray(attentions1),
                        np.asarray(attentions2), np.asarray(fmap_pos2))
    return loss, res


def kernel(x1_encoded, x2_encoded, attentions1, attentions2, fmap_pos2):
    loss, _ = _run(x1_encoded, x2_encoded, attentions1, attentions2,
                   fmap_pos2)
    return loss


# revision 23
# speedup vs baseline: 1.0508x; 1.0508x over previous
"""Trainium2 Bass kernel for nn_DistinctivenessLoss.

Math: with unit-normalized 48-dim descriptors, dist(a,b) < 1  <=>  <a,b> > 0.5,
so each image's two 4096x4096 cdist+count reductions collapse to ONE 4096x4096
gram matrix S = d1^T d2 per image:
  mx1[i] = #{j : S[i,j] > 0.5}          (row counts)
  mx2[i] = colcount[flat2[i]],  colcount[j] = #{i : S[i,j] > 0.5}

Sharding: 2 cores per image (B=4, 8 cores); each core computes 2048 rows of S.
On-device per core:
  - gram matmuls in bf16 (inputs rounded to bf16; fp32 PSUM accumulate --
    measured end-to-end loss impact ~4e-5 relative), row-tiled 2x (K=48<=64)
    at tile positions (0,0)/(64,0)
  - threshold pass split between ScalarE (Sign activation -> fp8 +-1, fused
    row-sum via accum_out) and VectorE (is_gt -> fp8 {0,1}, fused row-sum)
    over 2-bank [128,1024] PSUM supertiles
  - column counts via PE "selector" matmuls over the fp8 sign tiles with
    4-way column tiling: M=32 weight strips at tile_position (0, 32j) let 4
    matmuls stream 4 DIFFERENT sign tiles concurrently through separate
    XBUSes (K=128 each), accumulating into one PSUM bank; chunk c of slot j
    lands in PSUM partition 32j+c and the 4 partials are summed on the host
  - the peaky-loss max-pool path for one 64x64 attention map (3x3 avg pool as
    two banded matmuls, 33x33 max pool as log-step max chains + 32x32 DVE
    transposes); the avg33 branch folds algebraically into a host-side
    bilinear form u^T X u (sum of avg pools == bilinear form in the input)
Host: input normalization/layout, count decode, gathers, and the O(n) loss
epilogue (exact fp32/f64, negligible work).
"""

import numpy as np
import ml_dtypes

B, C, H, W = 4, 48, 64, 64
N = H * W                  # 4096
NCORES = 8
ROWS = N // 2              # rows of S per core
NT = ROWS // 128           # 16 row-tiles per core
NCH = N // 512             # 8 column chunks
NSC = NCH // 2             # 4 superchunks (2-bank PSUM supertiles)
THR = 0.5
TAU = 0.25
LAMBDA_PEAKY = 0.2
NPOOL = 32
NEG_INF = float("-inf")

# (row-tile, superchunk) -> engine class: 0=ACT/sign(+-1), 1=DVE/{0,2}.
ASSIGN = np.fromfunction(lambda t, s: (t + s) % 2, (NT, NSC), dtype=np.int64)
N_ACT_PER_CHUNK = np.array([(ASSIGN[:, c // 2] == 0).sum() for c in range(NCH)])
N_ACT_PER_TILE = (ASSIGN == 0).sum(axis=1)    # [NT], in superchunk units

_PROGRAM = None


def _build_program():
    from contextlib import ExitStack

    import concourse.bass as bass
    import concourse.mybir as mybir
    import concourse.tile as tile
    from concourse import bacc

    F32 = mybir.dt.float32
    BF16 = mybir.dt.bfloat16
    FP8 = mybir.dt.float8e4
    Sign = mybir.ActivationFunctionType.Sign
    ts, ds = bass.ts, bass.ds

    nc = bacc.Bacc("TRN2", target_bir_lowering=False, debug=False,
                   num_devices=NCORES)

    d1w_d = nc.dram_tensor("d1w", [128, NT * 64], BF16, kind="ExternalInput").ap()
    d2d_d = nc.dram_tensor("d2d", [128, N], BF16, kind="ExternalInput").ap()
    wsel_d = nc.dram_tensor("wsel", [128, NCH * 4 * 32], FP8,
                            kind="ExternalInput").ap()
    attn_d = nc.dram_tensor("attn", [64, 64], F32, kind="ExternalInput").ap()
    pk_d = nc.dram_tensor("pk", [64, 64], F32, kind="ExternalInput").ap()

    rowA_d = nc.dram_tensor("rowA", [128, NT], F32, kind="ExternalOutput").ap()
    rowD_d = nc.dram_tensor("rowD", [128, NT], F32, kind="ExternalOutput").ap()
    cols_d = nc.dram_tensor("cols", [128, 512], F32, kind="ExternalOutput").ap()
    pkout_d = nc.dram_tensor("pkout", [64, 1], F32, kind="ExternalOutput").ap()

    with tile.TileContext(nc) as tc, ExitStack() as ctx:
        sb = ctx.enter_context(tc.tile_pool(name="sb", bufs=1))
        sgn_pool = ctx.enter_context(tc.tile_pool(name="sgn", bufs=3))

        # ---- inputs: peaky constants first (tiny; unblock the early peaky
        # matmuls), then gram operands in first-use order ----
        pk = sb.tile([64, 64], F32)
        nc.sync.dma_start(pk[:], pk_d)
        attn = sb.tile([64, 64], F32)
        nc.sync.dma_start(attn[:], attn_d)

        d1w = sb.tile([128, NT * 64], BF16)
        d2d = sb.tile([128, N], BF16)
        nc.sync.dma_start(d1w[:, 0:128], d1w_d[:, 0:128])
        nc.sync.dma_start(d2d[:, 0:1024], d2d_d[:, 0:1024])
        nc.sync.dma_start(d1w[:, 128:1024], d1w_d[:, 128:1024])
        nc.sync.dma_start(d2d[:, 1024:2048], d2d_d[:, 1024:2048])
        nc.sync.dma_start(d2d[:, 2048:4096], d2d_d[:, 2048:4096])
        wsel = sb.tile([128, NCH * 4 * 32], FP8)
        nc.sync.dma_start(wsel[:], wsel_d)

        nthr = sb.tile([128, 1], F32)
        nc.vector.memset(nthr[:], -THR)

        rowaccA = sb.tile([128, NT * NSC], F32)
        rowaccD = sb.tile([128, NT * NSC], F32)
        nc.vector.memset(rowaccA[:], 0.0)
        nc.vector.memset(rowaccD[:], 0.0)

        # ---- PE warm-up: dummy matmuls with no DMA dependencies run during
        # the input-DMA window and start the HAM activity window early ----
        pk_pool = ctx.enter_context(
            tc.tile_pool(name="pkps", bufs=1, space="PSUM"))
        # (memsets on DVE: sub-us, so the warmup starts almost immediately)
        warm_w = sb.tile([64, 128], BF16)
        warm_x = sb.tile([64, 512], BF16)
        nc.vector.memset(warm_w[:], 1.0)
        nc.vector.memset(warm_x[:], 1.0)
        warm_ps = pk_pool.tile([128, 512], F32, tag="pk")
        for _ in range(5):
            nc.tensor.matmul(warm_ps[:], warm_w[0:64, :], warm_x[0:64, :],
                             start=True, stop=True)

        gram_pool = ctx.enter_context(tc.tile_pool(name="gram", bufs=1,
                                                   space="PSUM"))
        cs_pool = ctx.enter_context(tc.tile_pool(name="cs", bufs=1, space="PSUM"))
        colacc = cs_pool.tile([128, 512], F32, tag="ca")

        # peaky state built incrementally between gram iterations so the
        # serial PE->ACT->PE chain never stalls an engine queue
        pk_state = {}

        def peaky_step(step):
            if step == 0:      # p1 = X^T B3 (PE, after warmup, pre-main)
                p1 = pk_pool.tile([128, 512], F32, tag="pk")
                nc.tensor.matmul(p1[0:64, 0:64], attn[:], pk[:],
                                 start=True, stop=True)
                pk_state["p1"] = p1
            elif step == 1:    # ACT's first op; p1 ready ~2.5us
                s1 = sb.tile([64, 64], F32, tag="s1")
                nc.scalar.copy(s1[:], pk_state["p1"][0:64, 0:64])
                pk_state["s1"] = s1
            elif step == 2:    # p2 = s1^T B3 = B3 X B3 = sali (PE)
                p2 = pk_pool.tile([128, 512], F32, tag="pk")
                nc.tensor.matmul(p2[0:64, 0:64], pk_state["s1"][:], pk[:],
                                 start=True, stop=True)
                pk_state["p2"] = p2
            elif step == 3:
                sali = sb.tile([64, 64], F32, tag="sali")
                nc.scalar.copy(sali[:], pk_state["p2"][0:64, 0:64])
                pk_state["sali"] = sali
            elif step == 4:
                with nc.named_scope("peaky"):
                    sali = pk_state["sali"]

                    # log-step max chains on DVE (walrus rejects
                    # tensor_tensor on the Pool engine).
                    def max_chain(src_ap, tag):
                        pb = sb.tile([64, 128], F32, tag=tag + "pb")
                        nc.gpsimd.memset(pb[:, 0:16], NEG_INF)
                        nc.gpsimd.memset(pb[:, 80:112], NEG_INF)
                        nc.scalar.copy(pb[:, 16:80], src_ap)
                        cur = pb
                        for k in (1, 2, 4, 8, 16):
                            nxt = sb.tile([64, 128], F32, tag=tag + "s%d" % k)
                            nc.gpsimd.memset(nxt[:, 96:112], NEG_INF)
                            nc.vector.tensor_max(nxt[:, 0:96], cur[:, 0:96],
                                                 cur[:, k:96 + k])
                            cur = nxt
                        out = sb.tile([64, 64], F32, tag=tag + "o")
                        nc.vector.tensor_max(out[:], cur[:, 0:64],
                                             pb[:, 32:96])
                        return out

                    m1 = max_chain(sali[:], "mA")              # pooled along w
                    m1t = sb.tile([64, 64], F32, tag="m1t")
                    for bi in range(2):
                        for bj in range(2):
                            nc.vector.transpose(
                                m1t[ds(32 * bi, 32), ds(32 * bj, 32)],
                                m1[ds(32 * bj, 32), ds(32 * bi, 32)])
                    m2 = max_chain(m1t[:], "mB")               # pooled along h
                    gapm = sb.tile([64, 1], F32)
                    nc.vector.reduce_sum(gapm[:], m2[:],
                                         axis=mybir.AxisListType.X)
                    nc.sync.dma_start(pkout_d, gapm[:])

        peaky_step(0)
        peaky_step(1)

        # ---- main loop: gram + threshold + col-tiled colsum, software-
        # pipelined: superchunk sc's colsum matmuls are emitted while sc+1's
        # grams stream, so the PE always has independent work queued ----
        ncs = [0, 0, 0, 0]             # per-slot matmul counters
        n_cs = NCH * NT // 4           # 32 matmuls per column slot

        def colsum_mm(sc, cc, t, sgn):
            c = 2 * sc + cc
            j = t % 4
            w = wsel[:, ds((c * 4 + j) * 32, 32)]
            rhs = sgn[:, ds(t * 1024 + cc * 512, 512)]
            ncs[j] += 1
            nc.tensor.matmul(colacc[ds(32 * j, 32), :], w, rhs,
                             start=(ncs[j] == 1), stop=(ncs[j] == n_cs),
                             tile_position=(0, 32 * j),
                             skip_group_check=True)

        def colsum_batch(sc, sgn, sel):
            # 32 owed MMs per superchunk, ordered quartet-major so 4 matmuls
            # with distinct column slots (and distinct moving streams) are
            # adjacent and run concurrently; `sel` picks 4 of them
            for k in sel:
                q4, cc, j = k // 8, (k // 4) % 2, k % 4
                colsum_mm(sc, cc, 4 * q4 + j, sgn)

        peaky_after = {1: 2, 5: 3, 6: 4}   # global p-iter -> peaky step

        with nc.named_scope("main"):
            pending = None   # (sc, sgn) whose 32 colsum MMs are owed
            it = 0
            for sc in range(NSC):
                c0, c1 = 2 * sc, 2 * sc + 1
                sgn = sgn_pool.tile([128, NT * 1024], FP8, tag="sgn")
                for p in range(NT // 2):
                    g0 = gram_pool.tile([128, 1024], F32,
                                        tag="g%d" % ((2 * p) % 3))
                    g1 = gram_pool.tile([128, 1024], F32,
                                        tag="g%d" % ((2 * p + 1) % 3))
                    wl = d1w[0:C, ts(p, 128)]
                    wh = d1w[64:64 + C, ts(p, 128)]
                    nc.tensor.matmul(g0[:, 0:512], wl, d2d[0:C, ts(c0, 512)],
                                     start=True, stop=True)
                    nc.tensor.matmul(g0[:, 512:1024], wl, d2d[0:C, ts(c1, 512)],
                                     start=True, stop=True)
                    nc.tensor.matmul(g1[:, 0:512], wh,
                                     d2d[64:64 + C, ts(c0, 512)],
                                     start=True, stop=True)
                    nc.tensor.matmul(g1[:, 512:1024], wh,
                                     d2d[64:64 + C, ts(c1, 512)],
                                     start=True, stop=True)
                    for t, g in ((2 * p, g0), (2 * p + 1, g1)):
                        s = sgn[:, ts(t, 1024)]
                        if ASSIGN[t, sc] == 0:
                            nc.scalar.activation(
                                s, g[:], Sign, bias=nthr[:], scale=1.0,
                                accum_out=rowaccA[:, ds(t * NSC + sc, 1)])
                        else:
                            nc.vector.tensor_scalar(
                                s, g[:], THR, None,
                                mybir.AluOpType.is_gt, mybir.AluOpType.add,
                                accum_out=rowaccD[:, ds(t * NSC + sc, 1)])
                    if pending is not None:
                        psc, psgn = pending
                        colsum_batch(psc, psgn, range(4 * p, 4 * p + 4))
                    if it in peaky_after:
                        peaky_step(peaky_after[it])
                    it += 1
                pending = (sc, sgn)
            psc, psgn = pending
            colsum_batch(psc, psgn, range(32))

        # ---- row-count reduction + export ----
        redA = sb.tile([128, NT], F32)
        redD = sb.tile([128, NT], F32)
        nc.vector.reduce_sum(
            redA[:], rowaccA[:].rearrange("p (t c) -> p t c", t=NT, c=NSC),
            axis=mybir.AxisListType.X)
        nc.vector.reduce_sum(
            redD[:], rowaccD[:].rearrange("p (t c) -> p t c", t=NT, c=NSC),
            axis=mybir.AxisListType.X)
        nc.sync.dma_start(rowA_d, redA[:])
        nc.sync.dma_start(rowD_d, redD[:])

        # ---- column-count export: chunk c of slot j sits in partition
        # 32j+c; the 4 slot partials are summed on the host ----
        cols_sb = sb.tile([128, 512], F32)
        nc.scalar.copy(cols_sb[:], colacc[:])
        nc.sync.dma_start(cols_d, cols_sb[:])

    nc.compile()
    return nc


def _get_program():
    global _PROGRAM
    if _PROGRAM is None:
        _PROGRAM = _build_program()
    return _PROGRAM


def _normalize(x):
    n = np.sqrt((x * x).sum(axis=0, keepdims=True, dtype=np.float32))
    return (x / np.maximum(n, np.float32(1e-12))).astype(np.float32)


def _make_consts():
    idx = np.arange(64)
    b3 = (np.abs(idx[:, None] - idx[None, :]) <= 1).astype(np.float32) / \
        np.float32(3.0)
    b33 = (np.abs(idx[:, None] - idx[None, :]) <= 16).astype(np.float64) / \
        np.float64(33.0)
    # sum(avg33(sali)) == u^T X u with u = (B33 @ B3).sum(axis=0)
    u = (b33 @ b3.astype(np.float64)).sum(axis=0)          # [64] float64

    # col-tiled selector: block (c, j) is a [128, 32] weight whose column c
    # routes the colsum of chunk c (tiles t = j mod 4) into PSUM partition
    # 32j+c.  Slot j only ever sees tiles with t%2 == j%2, so the engine
    # class -- ACT sign (+-1, weight 0.5) vs DVE is_gt ({0,1}, weight 1.0)
    # -- is constant per block: class = (j + sc) % 2 with sc = c//2.
    wsel = np.zeros((128, NCH * 4 * 32), np.float32)
    for c in range(NCH):
        sc = c // 2
        for j in range(4):
            wsel[:, (c * 4 + j) * 32 + c] = 0.5 if (j + sc) % 2 == 0 else 1.0
    return b3, u, wsel.astype(ml_dtypes.float8_e4m3)


def _prepare_in_maps(x1, x2, a1, a2):
    b3, _, wsel = _make_consts()
    in_maps = []
    for b in range(B):
        d1 = _normalize(x1[b].reshape(C, N).astype(np.float32)) \
            .astype(ml_dtypes.bfloat16)
        d2 = _normalize(x2[b].reshape(C, N).astype(np.float32)) \
            .astype(ml_dtypes.bfloat16)
        d2d = np.zeros((128, N), ml_dtypes.bfloat16)
        d2d[0:C] = d2
        d2d[64:64 + C] = d2
        for half in range(2):
            base = half * ROWS
            d1w = np.zeros((128, NT * 64), ml_dtypes.bfloat16)
            for p in range(NT // 2):
                d1w[0:C, p * 128:(p + 1) * 128] = \
                    d1[:, base + (2 * p) * 128: base + (2 * p + 1) * 128]
                d1w[64:64 + C, p * 128:(p + 1) * 128] = \
                    d1[:, base + (2 * p + 1) * 128: base + (2 * p + 2) * 128]
            attn = (a1 if half == 0 else a2)[b, 0].astype(np.float32)
            in_maps.append({
                "d1w": d1w, "d2d": d2d, "wsel": np.asarray(wsel),
                "attn": np.ascontiguousarray(attn), "pk": b3,
            })
    return in_maps


def _postprocess(results, x1, x2, a1, a2, pos2):
    f32 = np.float32
    _, u, _ = _make_consts()
    mx1_halves = []
    colparts = []
    gap_means = np.zeros((NCORES,), np.float64)
    for core, r in enumerate(results):
        rowA = r["rowA"].astype(np.float32)   # [128, NT] sum of +-1 per tile
        rowD = r["rowD"].astype(np.float32)   # [128, NT] sum of {0,1}
        cnt = (rowA + 1024.0 * N_ACT_PER_TILE[None, :]) * 0.5 + rowD
        mx1_halves.append(cnt.T.reshape(ROWS))  # row t*128+p
        cols = r["cols"].astype(np.float32)     # [128, 512]: [4 slots, 32, 512]
        cols = cols.reshape(4, 32, 512)[:, 0:NCH, :].sum(axis=0)   # [NCH, 512]
        colpart = cols + 64.0 * N_ACT_PER_CHUNK[:, None]
        colparts.append(colpart.reshape(N))
        # peaky: mean gap = (sum max33(sali) - u^T X u) / 4096
        b = core // 2
        attn = (a1 if core % 2 == 0 else a2)[b, 0].astype(np.float64)
        uxu = u @ attn @ u
        m2sum = float(r["pkout"].astype(np.float64).sum())
        gap_means[core] = (m2sum - uxu) / float(N)

    loss_imgs = np.zeros((B,), np.float32)
    for b in range(B):
        mx1 = np.concatenate([mx1_halves[2 * b], mx1_halves[2 * b + 1]])
        colcnt = colparts[2 * b] + colparts[2 * b + 1]
        flat2 = (pos2[b, 0].astype(np.int64) * W +
                 pos2[b, 1].astype(np.int64))
        mx2 = colcnt[flat2]
        scores1 = a1[b].reshape(N).astype(np.float32)
        scores2 = a2[b].reshape(N).astype(np.float32)[flat2]
        t1 = (f32(1.0) / (f32(1.0) + mx1.astype(np.float32)) **
              f32(TAU)).astype(np.float32)
        t2 = (f32(1.0) / (f32(1.0) + mx2.astype(np.float32)) **
              f32(TAU)).astype(np.float32)
        loss_imgs[b] = (np.abs(scores1 - t1).mean(dtype=np.float32) +
                        np.abs(scores2 - t2).mean(dtype=np.float32))

    loss = loss_imgs.mean(dtype=np.float32)
    pk1 = max(f32(0.0), f32(1.0) - f32(gap_means[0::2].mean()))
    pk2 = max(f32(0.0), f32(1.0) - f32(gap_means[1::2].mean()))
    loss = loss + f32(LAMBDA_PEAKY) * (pk1 + pk2) / f32(2.0)
    return np.asarray(loss, dtype=np.float32)


def _run(x1_encoded, x2_encoded, attentions1, attentions2, fmap_pos2,
         trace=False, trace_cores=None):
    from concourse import bass_utils

    nc = _get_program()
    in_maps = _prepare_in_maps(np.asarray(x1_encoded), np.asarray(x2_encoded),
                               np.asarray(attentions1),
                               np.asarray(attentions2))
    res = bass_utils.run_bass_kernel_spmd(
        nc, in_maps, core_ids=list(range(NCORES)), trace=trace,
        trace_cores=trace_cores)
    loss = _postprocess(res.results, np.asarray(x1_encoded),
                        np.asarray(x2_encoded), np.asarray(attentions1),
                        np.asarray(attentions2), np.asarray(fmap_pos2))
    return loss, res


def kernel(x1_encoded, x2_encoded, attentions1, attentions2, fmap_pos2):
    loss, _ = _run(x1_encoded, x2_encoded, attentions1, attentions2,
                   fmap_pos2)
    return loss
